# revision 58
# baseline (speedup 1.0000x reference)
"""DiffAttention Trainium2 kernel.

Math (per batch b, head h):
  q,k split into two streams of D=64; v has E=128 channels.
  attn_s = softmax_causal(q_s k_s^T / 8) @ v            (s = 1,2)
  lam    = exp(lq1.lk1) - exp(lq2.lk2) + 0.8            (host scalar)
  x      = attn1 - lam*attn2
  out    = 0.2 * w * x * rsqrt(mean_e(x^2) + eps)

Structure (measured ~96us on 8 axon trn2 cores; prior best ~111us,
v1 baseline ~151us):
  - Stream-paired S matmuls: the two q/k streams live on SBUF partitions
    0:64 and 64:128, so their K=64 S-tile matmuls get tile_position
    (0,0) and (64,0) automatically. Issued adjacently, the PE runs them
    CONCURRENTLY in the two row-halves of the array (measured ~2x: a
    K=64 matmul alone runs at half rate, 0.95 ns/row vs 0.5 for K=128).
  - Causal-trimmed S matmuls and exps: diag-regime k-tiles (ki>4qb) only
    compute/exp q-cols >= (ki-4qb)*128. Saves ~15% PE rows + ACT elems.
  - Exp split ACT/DVE: every 3rd off-diag exp chunk runs on DVE via a
    bf16-Schraudolph bit-trick (i16 = x*A+B, bitcast bf16; B offset
    tuned for zero MEAN rel err so Schraudolph chunks don't bias the
    softmax); ACT (the bottleneck engine) keeps the rest.
  - PV with 4 one-bank u PSUM tags (one per stream x qi-parity; tag
    pairs qi and qi+2 sequentially -- one open accumulation group per
    2KB zero region): the epilogue reads u STRAIGHT FROM PSUM, deleting
    the 12.6us/core of PSUM->SBUF copy-out the old layout forced.
  - Drizzle schedule: each block's PV parts (<=4 k-tiles each) and its
    4 epilogue stage items are spread evenly across the NEXT block's
    S/exp slots in PE/DVE program order -- deps are then a whole block
    old, so the strict-FIFO engine queues never head-of-line stall
    (self-drizzling the current block measured ~8us WORSE).  Blocks run
    small-first then pairs interleaved by descending size, ending small
    (short head + short drain tail).
  - Epilogue: xb/xsq fused stts with accum_out for mean-square; rs =
    0.2*rsqrt(ms) as ACT Ln+Exp, with ONE explicit act-table load (set
    6 holds BOTH exp and ln -- otherwise the table pass thrashes
    0<->5 per block, 5x1.28us/iter); out-DMAs issued from gpsimd so
    SP's queue is inputs-only and prefetches a full iteration ahead.
"""

from contextlib import ExitStack

import numpy as np

import bass_rust as _bass_rust_mod
import concourse.bass as bass
import concourse.mybir as mybir


def _bass_rust_vec(pairs):
    return _bass_rust_mod.VecI64Pair([tuple(x) for x in pairs])
from concourse import bacc
from concourse._compat import axon_active
from concourse.bass import MemorySpace
from concourse.bass_utils import run_bass_kernel_spmd
from concourse.tile import TileContext

F32 = mybir.dt.float32
BF16 = mybir.dt.bfloat16
I16 = mybir.dt.int16
I32 = mybir.dt.int32
AF = mybir.ActivationFunctionType
ALU = mybir.AluOpType

B, L, H, D = 2, 2048, 8, 64
E = 2 * D               # 128 v-channels per head
NP = 128                # SBUF partitions
PAIRS = 2               # (b,h) pairs per core
N_CORES = 8
QB = 512                # q columns per block (4 tiles of 128)
NQB = L // QB           # 4
CHUNK = 2               # k-tiles exp'd per ACT op
NKT = L // NP           # 16 k tiles
LAMBDA_INIT = 0.8
EPS = 1e-5
OUT_SCALE = 1.0 - LAMBDA_INIT  # 0.2
SM_SCALE = 1.0 / 8.0

# bf16 Schraudolph: i16 = round(x*SM_SCALE*log2(e)*128 + (127-sigma)*128),
# bitcast as bf16 gives ~exp(x/8) with ~4% max rel error. The -1.75 bit
# offset zeroes the MEAN relative error (measured): a nonzero bias would
# systematically inflate Schraudolph chunks' softmax mass vs exact-exp
# chunks, which shows up directly in the output (~1e-2 at 1/3 coverage).
SCHRAU_A = float(128.0 * SM_SCALE * np.log2(np.e))
SCHRAU_B = float(128.0 * (127.0 - 0.0436775) - 1.75)


DVE_EVERY = 3  # every Nth off-diag exp chunk computed on DVE (0 = ACT only)
MASK_MERGE = False  # one affine_select per (block, stream) vs per diag chunk
PV_SPLIT = True  # split large PV accum groups into two drizzle items
U_COPY = False  # copy u PSUM->SBUF before epilogue vs read PSUM directly
OUT_DMA_ENGINE = "gpsimd"  # issue out-DMAs off the SP queue so next
#   iteration's input DMAs prefetch a full iteration ahead


def _build_program(w_is_ones: bool, repeat: int = 1, skip: frozenset = frozenset(),
                   dve_every: int = DVE_EVERY,
                   mask_merge: bool | None = None,
                   pv_split: bool | None = None,
                   u_copy_mode: bool | None = None,
                   out_dma_engine: str | None = None) -> bass.Bass:
    if mask_merge is None:
        mask_merge = MASK_MERGE
    if pv_split is None:
        pv_split = PV_SPLIT
    if u_copy_mode is None:
        u_copy_mode = U_COPY
    if out_dma_engine is None:
        out_dma_engine = OUT_DMA_ENGINE
    nc = bacc.Bacc(
        "TRN2",
        target_bir_lowering=False,
        debug=not axon_active(),
        enable_asserts=False,
        num_devices=N_CORES,
    )
    qt_d = nc.declare_dram_parameter("qt", [PAIRS, NP, L], BF16, isOutput=False)
    kt_d = nc.declare_dram_parameter("kt", [PAIRS, NP, L], BF16, isOutput=False)
    vv_d = nc.declare_dram_parameter("vv", [PAIRS, L, E + 1], BF16, isOutput=False)
    lam_d = nc.declare_dram_parameter("lam", [NP, 1], F32, isOutput=False)
    if not w_is_ones:
        wb_d = nc.declare_dram_parameter("wb", [NP, E], F32, isOutput=False)
    out_d = nc.declare_dram_parameter("out", [PAIRS, L, E], F32, isOutput=True)

    with TileContext(nc) as tc, ExitStack() as ctx:
        const = ctx.enter_context(tc.tile_pool(name="const", bufs=1))
        io = ctx.enter_context(tc.tile_pool(name="io", bufs=2))
        ptp = ctx.enter_context(tc.tile_pool(name="ptp", bufs=3))
        ep = ctx.enter_context(tc.tile_pool(name="ep", bufs=2))
        xp = ctx.enter_context(tc.tile_pool(name="xp", bufs=2))
        stp = ctx.enter_context(
            tc.tile_pool(name="stp", bufs=1, space=MemorySpace.PSUM)
        )
        up = ctx.enter_context(tc.tile_pool(name="up", bufs=1, space=MemorySpace.PSUM))

        lam_sb = const.tile([NP, 1], F32)
        nc.sync.dma_start(lam_sb[:], lam_d[:])
        # Force the activation table that contains BOTH Exp and Ln
        # (natural_log_exp_and_others, id 6) so the per-block Ln/Exp
        # finale never reloads tables: without this the table pass picks
        # exp_and_others for the exps and thrashes 0<->5 on every finale
        # (5 x 1.28us of in-loop ACT table loads).
        nc.scalar.add_instruction(mybir.InstLoadActFuncSet(act_func_set_id=6))
        ln02 = const.tile([NP, 1], F32)
        nc.gpsimd.memset(ln02[:], float(np.log(OUT_SCALE)))
        magic = const.tile([NP, 1], mybir.dt.int32)
        nc.gpsimd.memset(magic[:], 0x5F3759DF)
        if not w_is_ones:
            wb_sb = const.tile([NP, E], F32)
            nc.sync.dma_start(wb_sb[:], wb_d[:])

        st_par = [0]     # rotates the 2 st PSUM tags
        dve_ctr = [0]    # off-diag chunk counter for DVE assignment

        def exp_chunk(pt_dst, st_src, on_dve):
            """pt_dst (bf16 SBUF) = exp(SM_SCALE * st_src) (f32 PSUM)."""
            if "exp" in skip:
                nc.scalar.activation(
                    pt_dst[:, 0, 0:1], st_src[:, 0, 0:1], AF.Exp, scale=SM_SCALE
                )
                return
            if on_dve:
                nc.vector.tensor_scalar(
                    pt_dst.bitcast(I16), st_src, SCHRAU_A, SCHRAU_B,
                    ALU.mult, ALU.add,
                )
            else:
                nc.scalar.activation(pt_dst, st_src, AF.Exp, scale=SM_SCALE)

        rep_ctx = tc.For_i(0, repeat, 1) if repeat > 1 else None
        if rep_ctx is not None:
            ctx.enter_context(rep_ctx)

        def emit_pv_parts(c):
            """This block's PV matmuls as (ready_khi, item) pairs.

            A part covering k-tiles [klo, khi) only needs exps/masks up
            to khi, so it can run DURING this block's own A phase once
            chunk khi-1 is done (self-drizzle) -- every block then hosts
            exactly its own PV, and the PE never waits a whole block.
            """
            qb, pt, vv_sb = c["qb"], c["pt"], c["vv"]

            def u_tile(s, qi):
                # tag 'a' holds qi 0 (row 0) and 2 (row 1); tag 'b' holds
                # qi 1 and 3. A PSUM zero region (2KB bank) admits ONE
                # open accumulation group, and a tag is exactly one bank:
                # pairing (qi, qi+2) lets group qi complete (its last
                # k-tile is 4qb+qi) before group qi+2 must start.
                key = f"u{s}{'ab'[qi % 2]}"
                if c.get(key) is None:
                    c[key] = up.tile([NP, 2, 256], F32, tag=key, name=key)
                return c[key]

            def pv_part(s, qi, klo, khi):
                def f():
                    uh = u_tile(s, qi)
                    qt_g = 4 * qb + qi
                    for ki in range(klo, khi):
                        nc.tensor.matmul(
                            uh[:, qi // 2, 0:E + 1],
                            pt[:, s, ki, qi * NP:(qi + 1) * NP],
                            vv_sb[:, ki, :],
                            start=(ki == 0),
                            stop=(ki == qt_g),
                        )
                return f

            parts = []
            if "pv" not in skip:
                for qi in range(4):
                    n = 4 * qb + qi + 1
                    # the second group of a tag may only start after the
                    # first group (qi-2, ending at k-tile 4qb+qi-2)
                    # stopped
                    floor_ready = (4 * qb + qi - 1) if qi >= 2 else 0
                    for s in range(2):
                        klo = 0
                        while klo < n:
                            khi = min(klo + 4, n)
                            parts.append((max(khi, floor_ready),
                                          pv_part(s, qi, klo, khi)))
                            klo = khi
            parts.sort(key=lambda t: t[0])
            return parts

        dma_eng = {"gpsimd": nc.gpsimd, "sync": nc.sync,
                   "act": nc.scalar}[out_dma_engine]

        def emit_epilogue_items(c):
            """Per-block epilogue as drizzle stages, reading u from PSUM.

            Stages: A) t2 + eps-corr + xb, B) xsq+accum -> ms,
            C) ACT Ln/Exp finale -> rs = 0.2*rsqrt(ms), D) o = xb*rs
            + out-DMA. The single act-table (id 6) holds both Exp and
            Ln, so the finale costs no table loads.
            """
            p, qb = c["p"], c["qb"]

            def out_dma(o_qb):
                dma_eng.dma_start(
                    out_d[p].rearrange("(t q) e -> q t e", q=NP)[
                        :, 4 * qb:4 * qb + 4, :
                    ],
                    o_qb[:],
                )

            if "epi" in skip or "pv" in skip:
                def zero_out():
                    o_qb = xp.tile([NP, 4, E], F32, tag="o")
                    nc.gpsimd.memset(o_qb[:], 0.0)
                    out_dma(o_qb)
                return [zero_out]

            def u0(qi):
                return c[f"u0{'ab'[qi % 2]}"][:, qi // 2, :]

            def u1(qi):
                return c[f"u1{'ab'[qi % 2]}"][:, qi // 2, :]

            def di(qi):
                # position of qi in tag-major (den/t2/ms) order [0,2,1,3]
                return (qi % 2) * 2 + qi // 2

            def stage_a():
                xb = xp.tile([NP, 4, E], F32, tag="xb")
                c["xb"] = xb
                # DVE ops may read at most ONE non-scalar PSUM operand:
                # pull the 4 den scalars into SBUF first (tiny copies),
                # then every op below has a single PSUM input.
                den = ep.tile([NP, 2, 4], F32, tag="den")
                for s in range(2):
                    for h in range(2):
                        nc.vector.tensor_copy(
                            den[:, s, 2 * h:2 * h + 2],
                            c[f"u{s}{'ab'[h]}"][:, :, 0],
                        )
                c["den"] = den
                # t2 = lam*den1*U2 in one fused stt (lam per-partition
                # scalar, den1 broadcast along e); one op per qi-half
                # (the halves live in different PSUM tiles)
                t2 = ep.tile([NP, 4, E], F32, tag="t2")
                for h in range(2):
                    nc.vector.scalar_tensor_tensor(
                        t2[:, 2 * h:2 * h + 2, :],
                        c[f"u1{'ab'[h]}"][:, :, 1:E + 1],
                        lam_sb[:, 0:1],
                        den[:, 0, 2 * h:2 * h + 2].unsqueeze(2)
                           .broadcast_to([NP, 2, E]),
                        ALU.mult,
                        ALU.mult,
                    )
                # eps correction input: dd = sqrt(eps)*den1*den2
                dd = ep.tile([NP, 4], F32, tag="dd")
                nc.vector.scalar_tensor_tensor(
                    dd[:],
                    den[:, 0, :],
                    float(np.sqrt(EPS)),
                    den[:, 1, :],
                    ALU.mult,
                    ALU.mult,
                )
                c["dd"] = dd
                for qi in range(4):
                    nc.vector.scalar_tensor_tensor(
                        xb[:, qi, :],
                        u0(qi)[:, 1:E + 1],
                        den[:, 1, di(qi):di(qi) + 1],
                        t2[:, di(qi), :],
                        ALU.mult,
                        ALU.subtract,
                    )

            def stage_b():
                xb, dd = c["xb"], c["dd"]
                msb = ep.tile([NP, 4], F32, tag="msb")
                xsq = ep.tile([NP, 4, E], F32, tag="xsq")
                for qi in range(4):
                    # ms = mean(x^2) via accum_out on the square pass
                    # (msb kept in tag-major order to line up with dd)
                    nc.vector.scalar_tensor_tensor(
                        xsq[:, qi, :],
                        xb[:, qi, :],
                        1.0 / E,
                        xb[:, qi, :],
                        ALU.mult,
                        ALU.mult,
                        accum_out=msb[:, di(qi):di(qi) + 1],
                    )
                edd = ep.tile([NP, 4], F32, tag="edd")
                nc.vector.tensor_tensor(edd[:], dd[:], dd[:], ALU.mult)
                ms = ep.tile([NP, 4], F32, tag="msq")
                nc.vector.tensor_tensor(ms[:], msb[:], edd[:], ALU.add)
                c["ms"] = ms

            def stage_c():
                ms = c["ms"]
                rs = ep.tile([NP, 4], F32, tag="rs")
                if c.get("tail"):
                    # Tail blocks: their stage_c executes after the last
                    # exps, and on ACT it would sit in the FIFO ahead of
                    # the NEXT For_i iteration's first exp chunks,
                    # stalling ACT ~8us per iteration. Quake rsqrt + one
                    # Newton step on DVE instead (rsqrt rel err <=
                    # 1.8e-3); DVE's next-iteration work starts later,
                    # so its FIFO has slack here.
                    sh = ep.tile([NP, 4], I32, tag="qsh")
                    nc.vector.tensor_scalar(
                        sh[:], ms[:].bitcast(I32), 1, None,
                        ALU.logical_shift_right,
                    )
                    y = ep.tile([NP, 4], F32, tag="qy")
                    nc.vector.tensor_tensor(
                        y[:].bitcast(I32),
                        magic[:].broadcast_to([NP, 4]).bitcast(I32),
                        sh[:],
                        ALU.subtract,
                    )
                    mh = ep.tile([NP, 4], F32, tag="qmh")
                    nc.vector.tensor_scalar(mh[:], ms[:], 0.5, None,
                                            ALU.mult)
                    yy = ep.tile([NP, 4], F32, tag="qyy")
                    nc.vector.tensor_tensor(yy[:], y[:], y[:], ALU.mult)
                    nc.vector.tensor_tensor(yy[:], yy[:], mh[:], ALU.mult)
                    nc.vector.tensor_scalar(
                        yy[:], yy[:], 1.5, -OUT_SCALE,
                        ALU.subtract, ALU.mult,
                    )
                    nc.vector.tensor_tensor(rs[:], y[:], yy[:], ALU.mult)
                else:
                    # rs = exp(-0.5*ln(ms) + ln(0.2)) = 0.2*rsqrt(ms) on
                    # ACT; Ln and Exp share the forced table set 6 -> no
                    # reloads.
                    lg = ep.tile([NP, 4], F32, tag="lg")
                    nc.scalar.activation(lg[:], ms[:], AF.Ln)
                    nc.scalar.activation(rs[:], lg[:], AF.Exp,
                                         scale=-0.5, bias=ln02[:, 0:1])
                c["rs"] = rs

            def stage_d():
                xb, rs = c["xb"], c["rs"]
                o_qb = xp.tile([NP, 4, E], F32, tag="o")
                for qi in range(4):
                    # per-qi (rs is tag-major, o must be q-tile order)
                    nc.vector.tensor_tensor(
                        o_qb[:, qi, :],
                        xb[:, qi, :],
                        rs[:, di(qi):di(qi) + 1].broadcast_to([NP, E]),
                        ALU.mult,
                    )
                if not w_is_ones:
                    nc.vector.tensor_tensor(
                        o_qb[:],
                        o_qb[:],
                        wb_sb[:].unsqueeze(1).broadcast_to([NP, 4, E]),
                        ALU.mult,
                    )
                out_dma(o_qb)

            return [stage_a, stage_b, stage_c, stage_d]

        pending = []          # previous block's PV parts + epilogue stages
        emitted = [0]

        ios = []
        for p in range(PAIRS):
            qt_sb = io.tile([NP, L], BF16, tag=f"qt{p}", name="qt_sb")
            kt_sb = io.tile([NP, L], BF16, tag=f"kt{p}", name="kt_sb")
            vv_sb = io.tile([NP, NKT, E + 1], BF16, tag=f"vv{p}", name="vv_sb")
            # Split input DMAs so the first S matmul (qb=3, ki=0) can
            # start as soon as qt cols 1536: and kt cols 0:512 land.
            nc.sync.dma_start(kt_sb[:, 0:QB], kt_d[p, :, 0:QB])
            for qq in reversed(range(NQB)):
                nc.sync.dma_start(
                    qt_sb[:, qq * QB:(qq + 1) * QB],
                    qt_d[p, :, qq * QB:(qq + 1) * QB],
                )
            nc.sync.dma_start(kt_sb[:, QB:L], kt_d[p, :, QB:L])
            nc.sync.dma_start(vv_sb[:], vv_d[p].rearrange("(t k) e -> k t e", k=NP))
            ios.append((qt_sb, kt_sb, vv_sb))

        # Blocks in descending size, pairs interleaved: each A phase then
        # hosts the SAME-size PV of the other pair's previous block, so
        # the drizzled PE work matches the exp time it must cover; the
        # kernel tail is the smallest block's PV + epilogue. PV/epilogue
        # of the previous block are interleaved between S/exp chunks in
        # PROGRAM ORDER -- the PE queue is strict FIFO, so this is what
        # actually fills PE gaps while ACT works through the exps.
        # Block order: a SMALL block first (its A hosts no drizzle -- the
        # pipeline head -- so waste the least PE time there), then pairs
        # interleaved in descending size so each A phase hosts a
        # same-or-smaller PV, ending with the other small block whose PV
        # + epilogue form the (short) drain tail.
        order = [(0, 0)]
        for qb in reversed(range(1, NQB)):
            for p in range(PAIRS):
                order.append((qb, p))
        order.append((0, 1))
        for bi, (qb, p) in enumerate(order):
            if True:
                qt_sb, kt_sb, vv_sb = ios[p]
                nki = 4 * qb + 4
                pt = ptp.tile([NP, 2, nki, QB], BF16, tag="pt", name="pt")
                cur = {"p": p, "qb": qb, "vv": vv_sb, "pt": pt,
                       "tail": bi >= len(order) - 3}
                # ki processed in pairs: both S-chunk pairs issue
                # back-to-back (2 st tags), so the PE stream switches
                # between S weights and PV weights half as often --
                # LDWEIGHTS cannot overlap an in-flight full-K matmul,
                # so every S<->PV boundary costs un-hidden weight-load.
                for ki2 in range(0, nki, 2):
                    sts = []
                    for ki in (ki2, ki2 + 1):
                        c0 = max(0, ki - 4 * qb) * NP
                        st_par[0] = (st_par[0] + 1) % 2
                        st = stp.tile(
                            [NP, 2, QB], F32,
                            tag=f"st{st_par[0]}", name=f"st{st_par[0]}",
                        )
                        sts.append((st, c0))
                        if "s" not in skip:
                            for s in range(2):
                                sp = slice(s * D, (s + 1) * D)
                                # tile_position (64*s, 0): the streams
                                # run concurrently in the PE row-halves
                                nc.tensor.matmul(
                                    st[:, s, c0:QB],
                                    kt_sb[sp, ki * NP:(ki + 1) * NP],
                                    qt_sb[sp, qb * QB + c0:(qb + 1) * QB],
                                    start=True,
                                    stop=True,
                                )
                    for ki in (ki2, ki2 + 1):
                        st, c0 = sts[ki - ki2]
                        dve_ctr[0] += 1
                        on_dve = (dve_every > 0 and c0 == 0
                                  and (dve_ctr[0] % dve_every == 0))
                        exp_chunk(pt[:, :, ki, c0:], st[:, :, c0:], on_dve)
                        if ki >= 4 * qb and "mask" not in skip:
                            # this chunk holds q-tile (ki-4qb)'s diagonal
                            # subtile; mask it right away so self-drizzled
                            # PV parts reading it never wait long
                            cd = (ki - 4 * qb) * NP
                            sl = pt[:, :, ki, cd:cd + NP]
                            nc.gpsimd.affine_select(
                                sl, sl,
                                pattern=[[0, 2], [1, NP]],
                                compare_op=ALU.is_ge,
                                fill=0.0,
                                base=0,
                                channel_multiplier=-1,
                            )
                    # drizzle the PREVIOUS block's PV parts + epilogue
                    # stages proportionally across this A phase (their
                    # dependencies completed a whole block ago, so the
                    # PE FIFO never head-of-line blocks on a pending exp
                    # -- measured ~8us better than self-drizzling the
                    # current block's parts)
                    den_s = max(1, nki - 2)
                    target = min(
                        len(pending),
                        (len(pending) * (ki2 + 2) + den_s - 1) // den_s,
                    )
                    while emitted[0] < target:
                        pending[emitted[0]]()
                        emitted[0] += 1
                # flush any leftover items of the previous block
                while emitted[0] < len(pending):
                    pending[emitted[0]]()
                    emitted[0] += 1
                pending = [f for _, f in emit_pv_parts(cur)]
                pending.extend(emit_epilogue_items(cur))
                emitted[0] = 0

        # drain the last block's PV + epilogue
        while emitted[0] < len(pending):
            pending[emitted[0]]()
            emitted[0] += 1

    nc.compile()
    return nc


_PROGRAM_CACHE: dict = {}


def _get_program(w_is_ones: bool, repeat: int = 1,
                 dve_every: int = DVE_EVERY) -> bass.Bass:
    key = (w_is_ones, repeat, dve_every)
    if key not in _PROGRAM_CACHE:
        _PROGRAM_CACHE[key] = _build_program(w_is_ones, repeat,
                                             dve_every=dve_every)
    return _PROGRAM_CACHE[key]


def make_in_maps(query, key, value, lambda_q1, lambda_k1, lambda_q2, lambda_k2,
                 sub_norm_w):
    """Host-side shard/pack. Returns (in_maps, w_is_ones)."""
    query = np.asarray(query, dtype=np.float32)
    key = np.asarray(key, dtype=np.float32)
    value = np.asarray(value, dtype=np.float32)
    lam = float(
        np.exp(np.sum(np.float64(lambda_q1) * np.float64(lambda_k1)))
        - np.exp(np.sum(np.float64(lambda_q2) * np.float64(lambda_k2)))
        + LAMBDA_INIT
    )
    w = np.asarray(sub_norm_w, dtype=np.float32)
    w_is_ones = bool(np.all(w == 1.0))

    import ml_dtypes

    bf16 = ml_dtypes.bfloat16
    q5 = query.reshape(B, L, H, 2 * D)
    k5 = key.reshape(B, L, H, 2 * D)
    v4 = value.reshape(B, L, H, E)
    lam_arr = np.full((NP, 1), lam, dtype=np.float32)
    wb = np.broadcast_to(w[None, :], (NP, E)).copy() if not w_is_ones else None

    in_maps = []
    for c in range(N_CORES):
        qt = np.empty((PAIRS, NP, L), dtype=bf16)
        kt = np.empty((PAIRS, NP, L), dtype=bf16)
        vv = np.empty((PAIRS, L, E + 1), dtype=bf16)
        for p in range(PAIRS):
            f = c * PAIRS + p
            b, h = divmod(f, H)
            qt[p] = q5[b, :, h].T.astype(bf16)
            kt[p] = k5[b, :, h].T.astype(bf16)
            vv[p, :, 0] = 1.0
            vv[p, :, 1:] = v4[b, :, h].astype(bf16)
        m = {"qt": qt, "kt": kt, "vv": vv, "lam": lam_arr}
        if not w_is_ones:
            m["wb"] = wb
        in_maps.append(m)
    return in_maps, w_is_ones


def assemble_output(results) -> np.ndarray:
    out = np.empty((B, L, H * E), dtype=np.float32)
    for c in range(N_CORES):
        o = results[c]["out"]
        for p in range(PAIRS):
            f = c * PAIRS + p
            b, h = divmod(f, H)
            out[b, :, h * E: (h + 1) * E] = o[p].astype(np.float32)
    return out


def kernel(query, key, value, lambda_q1, lambda_k1, lambda_q2, lambda_k2,
           sub_norm_w, **_unused):
    in_maps, w_is_ones = make_in_maps(
        query, key, value, lambda_q1, lambda_k1, lambda_q2, lambda_k2, sub_norm_w
    )
    nc = _get_program(w_is_ones)
    res = run_bass_kernel_spmd(nc, in_maps, core_ids=list(range(N_CORES)))
    return assemble_output(res.results)



# revision 60
# speedup vs baseline: 1.0437x; 1.0437x over previous
"""DiffAttention Trainium2 kernel.

Math (per batch b, head h):
  q,k split into two streams of D=64; v has E=128 channels.
  attn_s = softmax_causal(q_s k_s^T / 8) @ v            (s = 1,2)
  lam    = exp(lq1.lk1) - exp(lq2.lk2) + 0.8            (host scalar)
  x      = attn1 - lam*attn2
  out    = 0.2 * w * x * rsqrt(mean_e(x^2) + eps)

Structure (measured ~96us on 8 axon trn2 cores; prior best ~111us,
v1 baseline ~151us):
  - Stream-paired S matmuls: the two q/k streams live on SBUF partitions
    0:64 and 64:128, so their K=64 S-tile matmuls get tile_position
    (0,0) and (64,0) automatically. Issued adjacently, the PE runs them
    CONCURRENTLY in the two row-halves of the array (measured ~2x: a
    K=64 matmul alone runs at half rate, 0.95 ns/row vs 0.5 for K=128).
  - Causal-trimmed S matmuls and exps: diag-regime k-tiles (ki>4qb) only
    compute/exp q-cols >= (ki-4qb)*128. Saves ~15% PE rows + ACT elems.
  - Exp split ACT/DVE: every 3rd off-diag exp chunk runs on DVE via a
    bf16-Schraudolph bit-trick (i16 = x*A+B, bitcast bf16; B offset
    tuned for zero MEAN rel err so Schraudolph chunks don't bias the
    softmax); ACT (the bottleneck engine) keeps the rest.
  - PV with 4 one-bank u PSUM tags (one per stream x qi-parity; tag
    pairs qi and qi+2 sequentially -- one open accumulation group per
    2KB zero region): the epilogue reads u STRAIGHT FROM PSUM, deleting
    the 12.6us/core of PSUM->SBUF copy-out the old layout forced.
  - Drizzle schedule: each block's PV parts (<=4 k-tiles each) and its
    4 epilogue stage items are spread evenly across the NEXT block's
    S/exp slots in PE/DVE program order -- deps are then a whole block
    old, so the strict-FIFO engine queues never head-of-line stall
    (self-drizzling the current block measured ~8us WORSE).  Blocks run
    small-first then pairs interleaved by descending size, ending small
    (short head + short drain tail).
  - Epilogue: xb/xsq fused stts with accum_out for mean-square; rs =
    0.2*rsqrt(ms) as ACT Ln+Exp, with ONE explicit act-table load (set
    6 holds BOTH exp and ln -- otherwise the table pass thrashes
    0<->5 per block, 5x1.28us/iter); out-DMAs issued from gpsimd so
    SP's queue is inputs-only and prefetches a full iteration ahead.
"""

from contextlib import ExitStack

import numpy as np

import bass_rust as _bass_rust_mod
import concourse.bass as bass
import concourse.mybir as mybir


def _bass_rust_vec(pairs):
    return _bass_rust_mod.VecI64Pair([tuple(x) for x in pairs])
from concourse import bacc
from concourse._compat import axon_active
from concourse.bass import MemorySpace
from concourse.bass_utils import run_bass_kernel_spmd
from concourse.tile import TileContext

F32 = mybir.dt.float32
BF16 = mybir.dt.bfloat16
I16 = mybir.dt.int16
I32 = mybir.dt.int32
AF = mybir.ActivationFunctionType
ALU = mybir.AluOpType

B, L, H, D = 2, 2048, 8, 64
E = 2 * D               # 128 v-channels per head
NP = 128                # SBUF partitions
PAIRS = 2               # (b,h) pairs per core
N_CORES = 8
QB = 512                # q columns per block (4 tiles of 128)
NQB = L // QB           # 4
CHUNK = 2               # k-tiles exp'd per ACT op
NKT = L // NP           # 16 k tiles
LAMBDA_INIT = 0.8
EPS = 1e-5
OUT_SCALE = 1.0 - LAMBDA_INIT  # 0.2
SM_SCALE = 1.0 / 8.0

# bf16 Schraudolph: i16 = round(x*SM_SCALE*log2(e)*128 + (127-sigma)*128),
# bitcast as bf16 gives ~exp(x/8) with ~4% max rel error. The -1.75 bit
# offset zeroes the MEAN relative error (measured): a nonzero bias would
# systematically inflate Schraudolph chunks' softmax mass vs exact-exp
# chunks, which shows up directly in the output (~1e-2 at 1/3 coverage).
SCHRAU_A = float(128.0 * SM_SCALE * np.log2(np.e))
SCHRAU_B = float(128.0 * (127.0 - 0.0436775) - 1.75)


DVE_EVERY = 3  # every Nth off-diag exp chunk computed on DVE (0 = ACT only)
MASK_MERGE = False  # one affine_select per (block, stream) vs per diag chunk
PV_SPLIT = True  # split large PV accum groups into two drizzle items
U_COPY = False  # copy u PSUM->SBUF before epilogue vs read PSUM directly
OUT_DMA_ENGINE = "gpsimd"  # issue out-DMAs off the SP queue so next
#   iteration's input DMAs prefetch a full iteration ahead


def _build_program(w_is_ones: bool, repeat: int = 1, skip: frozenset = frozenset(),
                   dve_every: int = DVE_EVERY,
                   mask_merge: bool | None = None,
                   pv_split: bool | None = None,
                   u_copy_mode: bool | None = None,
                   out_dma_engine: str | None = None) -> bass.Bass:
    if mask_merge is None:
        mask_merge = MASK_MERGE
    if pv_split is None:
        pv_split = PV_SPLIT
    if u_copy_mode is None:
        u_copy_mode = U_COPY
    if out_dma_engine is None:
        out_dma_engine = OUT_DMA_ENGINE
    nc = bacc.Bacc(
        "TRN2",
        target_bir_lowering=False,
        debug=not axon_active(),
        enable_asserts=False,
        num_devices=N_CORES,
    )
    qt_d = nc.declare_dram_parameter("qt", [PAIRS, NP, L], BF16, isOutput=False)
    kt_d = nc.declare_dram_parameter("kt", [PAIRS, NP, L], BF16, isOutput=False)
    vv_d = nc.declare_dram_parameter("vv", [PAIRS, NP, NKT, E + 1], BF16,
                                 isOutput=False)
    lam_d = nc.declare_dram_parameter("lam", [NP, 1], F32, isOutput=False)
    if not w_is_ones:
        wb_d = nc.declare_dram_parameter("wb", [NP, E], F32, isOutput=False)
    out_d = nc.declare_dram_parameter("out", [PAIRS, L, E], F32, isOutput=True)

    with TileContext(nc) as tc, ExitStack() as ctx:
        const = ctx.enter_context(tc.tile_pool(name="const", bufs=1))
        io = ctx.enter_context(tc.tile_pool(name="io", bufs=2))
        ptp = ctx.enter_context(tc.tile_pool(name="ptp", bufs=3))
        ep = ctx.enter_context(tc.tile_pool(name="ep", bufs=2))
        xp = ctx.enter_context(tc.tile_pool(name="xp", bufs=2))
        stp = ctx.enter_context(
            tc.tile_pool(name="stp", bufs=1, space=MemorySpace.PSUM)
        )
        up = ctx.enter_context(tc.tile_pool(name="up", bufs=1, space=MemorySpace.PSUM))

        lam_sb = const.tile([NP, 1], F32)
        nc.sync.dma_start(lam_sb[:], lam_d[:])
        # Force the activation table that contains BOTH Exp and Ln
        # (natural_log_exp_and_others, id 6) so the per-block Ln/Exp
        # finale never reloads tables: without this the table pass picks
        # exp_and_others for the exps and thrashes 0<->5 on every finale
        # (5 x 1.28us of in-loop ACT table loads).
        nc.scalar.add_instruction(mybir.InstLoadActFuncSet(act_func_set_id=6))
        ln02 = const.tile([NP, 1], F32)
        nc.gpsimd.memset(ln02[:], float(np.log(OUT_SCALE)))
        magic = const.tile([NP, 1], mybir.dt.int32)
        nc.gpsimd.memset(magic[:], 0x5F3759DF)
        if not w_is_ones:
            wb_sb = const.tile([NP, E], F32)
            nc.sync.dma_start(wb_sb[:], wb_d[:])

        st_par = [0]     # rotates the 2 st PSUM tags
        dve_ctr = [0]    # off-diag chunk counter for DVE assignment

        def exp_chunk(pt_dst, st_src, on_dve):
            """pt_dst (bf16 SBUF) = exp(SM_SCALE * st_src) (f32 PSUM)."""
            if "exp" in skip:
                nc.scalar.activation(
                    pt_dst[:, 0, 0:1], st_src[:, 0, 0:1], AF.Exp, scale=SM_SCALE
                )
                return
            if on_dve:
                nc.vector.tensor_scalar(
                    pt_dst.bitcast(I16), st_src, SCHRAU_A, SCHRAU_B,
                    ALU.mult, ALU.add,
                )
            else:
                nc.scalar.activation(pt_dst, st_src, AF.Exp, scale=SM_SCALE)

        rep_ctx = tc.For_i(0, repeat, 1) if repeat > 1 else None
        if rep_ctx is not None:
            ctx.enter_context(rep_ctx)

        def emit_pv_parts(c):
            """This block's PV matmuls as (ready_khi, item) pairs.

            A part covering k-tiles [klo, khi) only needs exps/masks up
            to khi, so it can run DURING this block's own A phase once
            chunk khi-1 is done (self-drizzle) -- every block then hosts
            exactly its own PV, and the PE never waits a whole block.
            """
            qb, pt, vv_sb = c["qb"], c["pt"], c["vv"]

            def u_tile(s, qi):
                # tag 'a' holds qi 0 (row 0) and 2 (row 1); tag 'b' holds
                # qi 1 and 3. A PSUM zero region (2KB bank) admits ONE
                # open accumulation group, and a tag is exactly one bank:
                # pairing (qi, qi+2) lets group qi complete (its last
                # k-tile is 4qb+qi) before group qi+2 must start.
                key = f"u{s}{'ab'[qi % 2]}"
                if c.get(key) is None:
                    c[key] = up.tile([NP, 2, 256], F32, tag=key, name=key)
                return c[key]

            def pv_part(s, qi, klo, khi):
                def f():
                    uh = u_tile(s, qi)
                    qt_g = 4 * qb + qi
                    for ki in range(klo, khi):
                        nc.tensor.matmul(
                            uh[:, qi // 2, 0:E + 1],
                            pt[:, s, ki, qi * NP:(qi + 1) * NP],
                            vv_sb[:, ki, :],
                            start=(ki == 0),
                            stop=(ki == qt_g),
                        )
                return f

            parts = []
            if "pv" not in skip:
                for qi in range(4):
                    n = 4 * qb + qi + 1
                    # the second group of a tag may only start after the
                    # first group (qi-2, ending at k-tile 4qb+qi-2)
                    # stopped
                    floor_ready = (4 * qb + qi - 1) if qi >= 2 else 0
                    for s in range(2):
                        klo = 0
                        while klo < n:
                            khi = min(klo + 4, n)
                            parts.append((max(khi, floor_ready),
                                          pv_part(s, qi, klo, khi)))
                            klo = khi
            parts.sort(key=lambda t: t[0])
            return parts

        dma_eng = {"gpsimd": nc.gpsimd, "sync": nc.sync,
                   "act": nc.scalar}[out_dma_engine]

        def emit_epilogue_items(c):
            """Per-block epilogue as drizzle stages, reading u from PSUM.

            Stages: A) t2 + eps-corr + xb, B) xsq+accum -> ms,
            C) ACT Ln/Exp finale -> rs = 0.2*rsqrt(ms), D) o = xb*rs
            + out-DMA. The single act-table (id 6) holds both Exp and
            Ln, so the finale costs no table loads.
            """
            p, qb = c["p"], c["qb"]

            def out_dma(o_qb):
                dma_eng.dma_start(
                    out_d[p].rearrange("(t q) e -> q t e", q=NP)[
                        :, 4 * qb:4 * qb + 4, :
                    ],
                    o_qb[:],
                )

            if "epi" in skip or "pv" in skip:
                def zero_out():
                    o_qb = xp.tile([NP, 4, E], F32, tag="o")
                    nc.gpsimd.memset(o_qb[:], 0.0)
                    out_dma(o_qb)
                return [zero_out]

            def u0(qi):
                return c[f"u0{'ab'[qi % 2]}"][:, qi // 2, :]

            def u1(qi):
                return c[f"u1{'ab'[qi % 2]}"][:, qi // 2, :]

            def di(qi):
                # position of qi in tag-major (den/t2/ms) order [0,2,1,3]
                return (qi % 2) * 2 + qi // 2

            def stage_a():
                xb = xp.tile([NP, 4, E], F32, tag="xb")
                c["xb"] = xb
                # DVE ops may read at most ONE non-scalar PSUM operand:
                # pull the 4 den scalars into SBUF first (tiny copies),
                # then every op below has a single PSUM input.
                den = ep.tile([NP, 2, 4], F32, tag="den")
                for s in range(2):
                    for h in range(2):
                        nc.vector.tensor_copy(
                            den[:, s, 2 * h:2 * h + 2],
                            c[f"u{s}{'ab'[h]}"][:, :, 0],
                        )
                c["den"] = den
                # t2 = lam*den1*U2 in one fused stt (lam per-partition
                # scalar, den1 broadcast along e); one op per qi-half
                # (the halves live in different PSUM tiles)
                t2 = ep.tile([NP, 4, E], F32, tag="t2")
                for h in range(2):
                    nc.vector.scalar_tensor_tensor(
                        t2[:, 2 * h:2 * h + 2, :],
                        c[f"u1{'ab'[h]}"][:, :, 1:E + 1],
                        lam_sb[:, 0:1],
                        den[:, 0, 2 * h:2 * h + 2].unsqueeze(2)
                           .broadcast_to([NP, 2, E]),
                        ALU.mult,
                        ALU.mult,
                    )
                # eps correction input: dd = sqrt(eps)*den1*den2
                dd = ep.tile([NP, 4], F32, tag="dd")
                nc.vector.scalar_tensor_tensor(
                    dd[:],
                    den[:, 0, :],
                    float(np.sqrt(EPS)),
                    den[:, 1, :],
                    ALU.mult,
                    ALU.mult,
                )
                c["dd"] = dd
                for qi in range(4):
                    nc.vector.scalar_tensor_tensor(
                        xb[:, qi, :],
                        u0(qi)[:, 1:E + 1],
                        den[:, 1, di(qi):di(qi) + 1],
                        t2[:, di(qi), :],
                        ALU.mult,
                        ALU.subtract,
                    )

            def stage_b():
                xb, dd = c["xb"], c["dd"]
                msb = ep.tile([NP, 4], F32, tag="msb")
                xsq = ep.tile([NP, 4, E], F32, tag="xsq")
                for qi in range(4):
                    # ms = mean(x^2) via accum_out on the square pass
                    # (msb kept in tag-major order to line up with dd)
                    nc.vector.scalar_tensor_tensor(
                        xsq[:, qi, :],
                        xb[:, qi, :],
                        1.0 / E,
                        xb[:, qi, :],
                        ALU.mult,
                        ALU.mult,
                        accum_out=msb[:, di(qi):di(qi) + 1],
                    )
                edd = ep.tile([NP, 4], F32, tag="edd")
                nc.vector.tensor_tensor(edd[:], dd[:], dd[:], ALU.mult)
                ms = ep.tile([NP, 4], F32, tag="msq")
                nc.vector.tensor_tensor(ms[:], msb[:], edd[:], ALU.add)
                c["ms"] = ms

            def stage_c():
                ms = c["ms"]
                rs = ep.tile([NP, 4], F32, tag="rs")
                if c.get("tail"):
                    # Tail blocks: their stage_c executes after the last
                    # exps, and on ACT it would sit in the FIFO ahead of
                    # the NEXT For_i iteration's first exp chunks,
                    # stalling ACT ~8us per iteration. Quake rsqrt + one
                    # Newton step on DVE instead (rsqrt rel err <=
                    # 1.8e-3); DVE's next-iteration work starts later,
                    # so its FIFO has slack here.
                    sh = ep.tile([NP, 4], I32, tag="qsh")
                    nc.vector.tensor_scalar(
                        sh[:], ms[:].bitcast(I32), 1, None,
                        ALU.logical_shift_right,
                    )
                    y = ep.tile([NP, 4], F32, tag="qy")
                    nc.vector.tensor_tensor(
                        y[:].bitcast(I32),
                        magic[:].broadcast_to([NP, 4]).bitcast(I32),
                        sh[:],
                        ALU.subtract,
                    )
                    mh = ep.tile([NP, 4], F32, tag="qmh")
                    nc.vector.tensor_scalar(mh[:], ms[:], 0.5, None,
                                            ALU.mult)
                    yy = ep.tile([NP, 4], F32, tag="qyy")
                    nc.vector.tensor_tensor(yy[:], y[:], y[:], ALU.mult)
                    nc.vector.tensor_tensor(yy[:], yy[:], mh[:], ALU.mult)
                    nc.vector.tensor_scalar(
                        yy[:], yy[:], 1.5, -OUT_SCALE,
                        ALU.subtract, ALU.mult,
                    )
                    nc.vector.tensor_tensor(rs[:], y[:], yy[:], ALU.mult)
                else:
                    # rs = exp(-0.5*ln(ms) + ln(0.2)) = 0.2*rsqrt(ms) on
                    # ACT; Ln and Exp share the forced table set 6 -> no
                    # reloads.
                    lg = ep.tile([NP, 4], F32, tag="lg")
                    nc.scalar.activation(lg[:], ms[:], AF.Ln)
                    nc.scalar.activation(rs[:], lg[:], AF.Exp,
                                         scale=-0.5, bias=ln02[:, 0:1])
                c["rs"] = rs

            def stage_d():
                xb, rs = c["xb"], c["rs"]
                o_qb = xp.tile([NP, 4, E], F32, tag="o")
                for qi in range(4):
                    # per-qi (rs is tag-major, o must be q-tile order)
                    nc.vector.tensor_tensor(
                        o_qb[:, qi, :],
                        xb[:, qi, :],
                        rs[:, di(qi):di(qi) + 1].broadcast_to([NP, E]),
                        ALU.mult,
                    )
                if not w_is_ones:
                    nc.vector.tensor_tensor(
                        o_qb[:],
                        o_qb[:],
                        wb_sb[:].unsqueeze(1).broadcast_to([NP, 4, E]),
                        ALU.mult,
                    )
                out_dma(o_qb)

            return [stage_a, stage_b, stage_c, stage_d]

        pending = []          # previous block's PV parts + epilogue stages
        emitted = [0]

        ios = []
        for p in range(PAIRS):
            qt_sb = io.tile([NP, L], BF16, tag=f"qt{p}", name="qt_sb")
            kt_sb = io.tile([NP, L], BF16, tag=f"kt{p}", name="kt_sb")
            vv_sb = io.tile([NP, NKT, E + 1], BF16, tag=f"vv{p}", name="vv_sb")
            # Split input DMAs so the first S matmul (qb=3, ki=0) can
            # start as soon as qt cols 1536: and kt cols 0:512 land.
            nc.sync.dma_start(kt_sb[:, 0:QB], kt_d[p, :, 0:QB])
            for qq in reversed(range(NQB)):
                nc.sync.dma_start(
                    qt_sb[:, qq * QB:(qq + 1) * QB],
                    qt_d[p, :, qq * QB:(qq + 1) * QB],
                )
            nc.sync.dma_start(kt_sb[:, QB:L], kt_d[p, :, QB:L])
            # vv pre-transposed on the host to [k, t, e]: the DMA is one
            # contiguous 4128B line per partition (128 descriptors, not
            # 2048 -- SWDGE issue drops ~3.2us -> ~0.5us per pair)
            nc.sync.dma_start(vv_sb[:], vv_d[p])
            ios.append((qt_sb, kt_sb, vv_sb))

        # Blocks in descending size, pairs interleaved: each A phase then
        # hosts the SAME-size PV of the other pair's previous block, so
        # the drizzled PE work matches the exp time it must cover; the
        # kernel tail is the smallest block's PV + epilogue. PV/epilogue
        # of the previous block are interleaved between S/exp chunks in
        # PROGRAM ORDER -- the PE queue is strict FIFO, so this is what
        # actually fills PE gaps while ACT works through the exps.
        # Block order: a SMALL block first (its A hosts no drizzle -- the
        # pipeline head -- so waste the least PE time there), then pairs
        # interleaved in descending size so each A phase hosts a
        # same-or-smaller PV, ending with the other small block whose PV
        # + epilogue form the (short) drain tail.
        order = [(0, 0)]
        for qb in reversed(range(1, NQB)):
            for p in range(PAIRS):
                order.append((qb, p))
        order.append((0, 1))
        for bi, (qb, p) in enumerate(order):
            if True:
                qt_sb, kt_sb, vv_sb = ios[p]
                nki = 4 * qb + 4
                pt = ptp.tile([NP, 2, nki, QB], BF16, tag="pt", name="pt")
                cur = {"p": p, "qb": qb, "vv": vv_sb, "pt": pt,
                       "tail": False}  # quake-tail measured ~3.5us WORSE
                # ki processed in pairs: both S-chunk pairs issue
                # back-to-back (2 st tags), so the PE stream switches
                # between S weights and PV weights half as often --
                # LDWEIGHTS cannot overlap an in-flight full-K matmul,
                # so every S<->PV boundary costs un-hidden weight-load.
                for ki2 in range(0, nki, 2):
                    sts = []
                    for ki in (ki2, ki2 + 1):
                        c0 = max(0, ki - 4 * qb) * NP
                        st_par[0] = (st_par[0] + 1) % 2
                        st = stp.tile(
                            [NP, 2, QB], F32,
                            tag=f"st{st_par[0]}", name=f"st{st_par[0]}",
                        )
                        sts.append((st, c0))
                        if "s" not in skip:
                            for s in range(2):
                                sp = slice(s * D, (s + 1) * D)
                                # tile_position (64*s, 0): the streams
                                # run concurrently in the PE row-halves
                                nc.tensor.matmul(
                                    st[:, s, c0:QB],
                                    kt_sb[sp, ki * NP:(ki + 1) * NP],
                                    qt_sb[sp, qb * QB + c0:(qb + 1) * QB],
                                    start=True,
                                    stop=True,
                                )
                    for ki in (ki2, ki2 + 1):
                        st, c0 = sts[ki - ki2]
                        dve_ctr[0] += 1
                        on_dve = (dve_every > 0 and c0 == 0
                                  and (dve_ctr[0] % dve_every == 0))
                        exp_chunk(pt[:, :, ki, c0:], st[:, :, c0:], on_dve)
                        if ki >= 4 * qb and "mask" not in skip:
                            # this chunk holds q-tile (ki-4qb)'s diagonal
                            # subtile; mask it right away so self-drizzled
                            # PV parts reading it never wait long
                            cd = (ki - 4 * qb) * NP
                            sl = pt[:, :, ki, cd:cd + NP]
                            nc.gpsimd.affine_select(
                                sl, sl,
                                pattern=[[0, 2], [1, NP]],
                                compare_op=ALU.is_ge,
                                fill=0.0,
                                base=0,
                                channel_multiplier=-1,
                            )
                    # drizzle the PREVIOUS block's PV parts + epilogue
                    # stages proportionally across this A phase (their
                    # dependencies completed a whole block ago, so the
                    # PE FIFO never head-of-line blocks on a pending exp
                    # -- measured ~8us better than self-drizzling the
                    # current block's parts)
                    den_s = max(1, nki - 2)
                    target = min(
                        len(pending),
                        (len(pending) * (ki2 + 2) + den_s - 1) // den_s,
                    )
                    while emitted[0] < target:
                        pending[emitted[0]]()
                        emitted[0] += 1
                # flush any leftover items of the previous block
                while emitted[0] < len(pending):
                    pending[emitted[0]]()
                    emitted[0] += 1
                pending = [f for _, f in emit_pv_parts(cur)]
                pending.extend(emit_epilogue_items(cur))
                emitted[0] = 0

        # drain the last block's PV + epilogue
        while emitted[0] < len(pending):
            pending[emitted[0]]()
            emitted[0] += 1

    nc.compile()
    return nc


_PROGRAM_CACHE: dict = {}


def _get_program(w_is_ones: bool, repeat: int = 1,
                 dve_every: int = DVE_EVERY) -> bass.Bass:
    key = (w_is_ones, repeat, dve_every)
    if key not in _PROGRAM_CACHE:
        _PROGRAM_CACHE[key] = _build_program(w_is_ones, repeat,
                                             dve_every=dve_every)
    return _PROGRAM_CACHE[key]


def make_in_maps(query, key, value, lambda_q1, lambda_k1, lambda_q2, lambda_k2,
                 sub_norm_w):
    """Host-side shard/pack. Returns (in_maps, w_is_ones)."""
    query = np.asarray(query, dtype=np.float32)
    key = np.asarray(key, dtype=np.float32)
    value = np.asarray(value, dtype=np.float32)
    lam = float(
        np.exp(np.sum(np.float64(lambda_q1) * np.float64(lambda_k1)))
        - np.exp(np.sum(np.float64(lambda_q2) * np.float64(lambda_k2)))
        + LAMBDA_INIT
    )
    w = np.asarray(sub_norm_w, dtype=np.float32)
    w_is_ones = bool(np.all(w == 1.0))

    import ml_dtypes

    bf16 = ml_dtypes.bfloat16
    q5 = query.reshape(B, L, H, 2 * D)
    k5 = key.reshape(B, L, H, 2 * D)
    v4 = value.reshape(B, L, H, E)
    lam_arr = np.full((NP, 1), lam, dtype=np.float32)
    wb = np.broadcast_to(w[None, :], (NP, E)).copy() if not w_is_ones else None

    in_maps = []
    for c in range(N_CORES):
        qt = np.empty((PAIRS, NP, L), dtype=bf16)
        kt = np.empty((PAIRS, NP, L), dtype=bf16)
        vv = np.empty((PAIRS, NP, NKT, E + 1), dtype=bf16)
        for p in range(PAIRS):
            f = c * PAIRS + p
            b, h = divmod(f, H)
            qt[p] = q5[b, :, h].T.astype(bf16)
            kt[p] = k5[b, :, h].T.astype(bf16)
            vv[p, :, :, 0] = 1.0
            vv[p, :, :, 1:] = v4[b, :, h].reshape(NKT, NP, E).transpose(
                1, 0, 2).astype(bf16)
        m = {"qt": qt, "kt": kt, "vv": vv, "lam": lam_arr}
        if not w_is_ones:
            m["wb"] = wb
        in_maps.append(m)
    return in_maps, w_is_ones


def assemble_output(results) -> np.ndarray:
    out = np.empty((B, L, H * E), dtype=np.float32)
    for c in range(N_CORES):
        o = results[c]["out"]
        for p in range(PAIRS):
            f = c * PAIRS + p
            b, h = divmod(f, H)
            out[b, :, h * E: (h + 1) * E] = o[p].astype(np.float32)
    return out


def kernel(query, key, value, lambda_q1, lambda_k1, lambda_q2, lambda_k2,
           sub_norm_w, **_unused):
    in_maps, w_is_ones = make_in_maps(
        query, key, value, lambda_q1, lambda_k1, lambda_q2, lambda_k2, sub_norm_w
    )
    nc = _get_program(w_is_ones)
    res = run_bass_kernel_spmd(nc, in_maps, core_ids=list(range(N_CORES)))
    return assemble_output(res.results)

